# revision 1
# baseline (speedup 1.0000x reference)
"""CfC Liquid Cell kernel for Trainium2 (Bass/Tile), 8 NeuronCores.

Sharding: data-parallel over batch (B=8 -> 1 batch element per core).
Per-core plan (S=2048, H=1024, NH=16, HD=64, NS=64, K=4):

Phase A (chunks of TA=512 over time):
  - DMA x chunk (time-major), PE-transpose to feature-major x^T
  - in_proj matmuls (weights stationary, x^T moving) -> xz^T in PSUM
  - x_path half copied to SBUF, z half silu'd; both stored to DRAM scratch

Phase B (chunks of TB=256 over time):
  - depthwise causal conv = 4 shifted diagonal matmuls + bias "tap" on PE,
    accumulated in PSUM; silu on ACT
  - head matmuls (bb/f1/f2/tau/decay/state_out) with 2-head block-diagonal
    64x64 weights -> full 128-partition tiles
  - all activations via tanh/silu only (single ACT table set):
    sigmoid(u) = 0.5 + 0.5*tanh(u/2)
  - h_t = d_t*h_{t-1} + (1-d_t)*c_t via DVE tensor_tensor_scan
  - out_proj with gated activations as the *stationary* operand -> y is
    produced time-major, DMA'd straight out (no output transpose)
"""

import numpy as np

B, S, H = 8, 2048, 1024
NH, HD, NS, K = 16, 64, 64, 4
N_CORES = 8
TA = 512            # phase A time chunk
TB = 256            # phase B time chunk
NA = S // TA        # 4
NB = S // TB        # 8
P = 128

_CACHE = {}


def _build_program():
    import concourse.bacc as bacc
    import concourse.mybir as mybir
    import concourse.tile as tile

    F32 = mybir.dt.float32
    F32R = mybir.dt.float32r
    AF = mybir.ActivationFunctionType
    ALU = mybir.AluOpType

    nc = bacc.Bacc("TRN2", target_bir_lowering=False, debug=False)

    x_d = nc.dram_tensor("x", (S, H), F32, kind="ExternalInput").ap()
    w_in_d = nc.dram_tensor("w_in", (P, 8, 2 * H), F32R, kind="ExternalInput").ap()
    w_out_d = nc.dram_tensor("w_out", (P, 8, H), F32R, kind="ExternalInput").ap()
    cdiag_d = nc.dram_tensor("cdiag", (P, 8, 5, P), F32R, kind="ExternalInput").ap()
    blk_d = nc.dram_tensor("blk", (P, 6, P), F32R, kind="ExternalInput").ap()
    bias_d = nc.dram_tensor("bias", (P, 6), F32, kind="ExternalInput").ap()
    ident_d = nc.dram_tensor("ident", (P, P), F32, kind="ExternalInput").ap()
    y_d = nc.dram_tensor("y", (S, H), F32, kind="ExternalOutput").ap()

    with tile.TileContext(nc) as tc:
        # DRAM scratch for the phase A -> phase B handoff (feature-major)
        with tc.tile_pool(name="dram", bufs=1, space="DRAM") as dpool:
            xh_d = dpool.tile([8, P, S], F32R)  # silu(conv(x_path))^T
            zs_d = dpool.tile([8, P, S], F32)   # silu(z)^T

            # ---------------- Phase A ----------------
            # transpose -> in_proj -> {x_path -> conv -> silu -> xh, z -> silu}
            with tc.tile_pool(name="ca", bufs=1) as ca:
                w_in = ca.tile([P, 8, 2 * H], F32R)
                nc.sync.dma_start(w_in[:], w_in_d[:])
                cdiag = ca.tile([P, 8, 5, P], F32R)
                nc.sync.dma_start(cdiag[:], cdiag_d[:])
                ident = ca.tile([P, P], F32)
                nc.sync.dma_start(ident[:], ident_d[:])
                onesA = ca.tile([P, TA], F32R)
                nc.vector.memset(onesA[:].bitcast(F32), 1.0)

                with tc.tile_pool(name="pax", bufs=1) as pax, \
                     tc.tile_pool(name="paxT", bufs=1) as paxT, \
                     tc.tile_pool(name="paxp", bufs=2) as paxp, \
                     tc.tile_pool(name="paout", bufs=2) as paout, \
                     tc.tile_pool(name="pazs", bufs=1) as pazs, \
                     tc.tile_pool(name="patr", bufs=4, space="PSUM") as patr, \
                     tc.tile_pool(name="pamm", bufs=3, space="PSUM") as pamm:
                    xp_prev = None
                    for c in range(NA):
                        xc = pax.tile([P, TA // P, H], F32, tag="xc")
                        nc.sync.dma_start(
                            xc[:],
                            x_d[c * TA:(c + 1) * TA, :].rearrange(
                                "(ss p) k -> p ss k", p=P))
                        xT = paxT.tile([P, 8, TA], F32R, tag="xT")
                        for ss in range(TA // P):
                            for kt in range(8):
                                pt = patr.tile([P, P], F32, tag="tr")
                                nc.tensor.transpose(
                                    pt[:], xc[:, ss, kt * P:(kt + 1) * P], ident[:])
                                if kt % 2 == 0:
                                    nc.vector.tensor_copy(
                                        xT[:, kt, ss * P:(ss + 1) * P], pt[:])
                                else:
                                    nc.scalar.activation(
                                        xT[:, kt, ss * P:(ss + 1) * P], pt[:], AF.Copy)
                        # x_path buffer with 3-column causal halo
                        xp = paxp.tile([P, 8, 3 + TA], F32R, tag="xp")
                        if c == 0:
                            nc.vector.memset(xp[:, :, :3].bitcast(F32), 0.0)
                        else:
                            nc.vector.tensor_copy(xp[:, :, :3], xp_prev[:, :, TA:TA + 3])
                        zs = pazs.tile([P, 8, TA], F32, tag="zs")
                        for jt in range(16):
                            pm = pamm.tile([P, TA], F32, tag="mm")
                            for kt in range(8):
                                nc.tensor.matmul(
                                    pm[:], w_in[:, kt, jt * P:(jt + 1) * P],
                                    xT[:, kt, :],
                                    start=(kt == 0), stop=(kt == 7))
                            if jt < 8:
                                nc.vector.tensor_copy(xp[:, jt, 3:], pm[:])
                            else:
                                nc.scalar.activation(zs[:, jt - 8, :], pm[:], AF.Silu)
                        xp_prev = xp
                        # depthwise causal conv: 4 shifted diag taps + bias tap
                        xh = paout.tile([P, 8, TA], F32R, tag="xh")
                        for ct in range(8):
                            pc = pamm.tile([P, TA], F32, tag="mm")
                            for tap in range(4):
                                nc.tensor.matmul(
                                    pc[:], cdiag[:, ct, tap, :],
                                    xp[:, ct, tap:tap + TA],
                                    start=(tap == 0), stop=False)
                            nc.tensor.matmul(
                                pc[:], cdiag[:, ct, 4, :], onesA[:],
                                start=False, stop=True)
                            nc.scalar.activation(xh[:, ct, :], pc[:], AF.Silu)
                        nc.sync.dma_start(
                            xh_d[:, :, c * TA:(c + 1) * TA].transpose([1, 0, 2]), xh[:])
                        nc.sync.dma_start(
                            zs_d[:, :, c * TA:(c + 1) * TA].transpose([1, 0, 2]), zs[:])

            # ---------------- Phase B ----------------
            with tc.tile_pool(name="cb", bufs=1) as cb:
                w_out = cb.tile([P, 8, H], F32R)
                nc.sync.dma_start(w_out[:], w_out_d[:])
                blk = cb.tile([P, 6, P], F32R)
                nc.sync.dma_start(blk[:], blk_d[:])
                bias = cb.tile([P, 6], F32)
                nc.sync.dma_start(bias[:], bias_d[:])

                with tc.tile_pool(name="pbs", bufs=2) as pbs, \
                     tc.tile_pool(name="work", bufs=22) as work, \
                     tc.tile_pool(name="ph", bufs=2) as ph, \
                     tc.tile_pool(name="py", bufs=2) as pyp, \
                     tc.tile_pool(name="psst", bufs=6, space="PSUM") as psst, \
                     tc.tile_pool(name="psy", bufs=1, space="PSUM") as psy:

                    def stage(widx, rhs_t, rhs_off, out_t, func, bias_col):
                        for q in range(2):
                            pg = psst.tile([P, 2, TB], F32, tag="stage", name="pg")
                            nc.tensor.matmul(
                                pg[:], blk[:, widx, :],
                                rhs_t[:, rhs_off + 2 * q:rhs_off + 2 * q + 2, :],
                                start=True, stop=True)
                            nc.scalar.activation(
                                out_t[:, 2 * q:2 * q + 2, :], pg[:], func,
                                bias=bias[:, bias_col:bias_col + 1])

                    def s12_heads_chain(c, xhc, h_prev):
                        """gate matmuls + activations + scan chain for chunk c"""
                        h = ph.tile([P, 8, TB], F32R, tag="h", name="h")
                        for hf in range(2):
                            p0 = 4 * hf
                            bbh = work.tile([P, 4, TB], F32R, tag="work", name="bbh")
                            stage(0, xhc, p0, bbh, AF.Silu, 0)
                            f1 = work.tile([P, 4, TB], F32, tag="work", name="f1")
                            stage(1, bbh, 0, f1, AF.Tanh, 1)
                            f2 = work.tile([P, 4, TB], F32, tag="work", name="f2")
                            stage(2, bbh, 0, f2, AF.Tanh, 2)
                            ttau = work.tile([P, 4, TB], F32, tag="work", name="ttau")
                            stage(3, bbh, 0, ttau, AF.Tanh, 3)
                            td = work.tile([P, 4, TB], F32, tag="work", name="td")
                            stage(4, bbh, 0, td, AF.Tanh, 4)

                            tau = work.tile([P, 4, TB], F32, tag="work", name="tau")
                            nc.vector.tensor_scalar(tau[:], ttau[:], 0.5, 0.5,
                                                    ALU.mult, ALU.add)
                            dd = work.tile([P, 4, TB], F32, tag="work", name="dd")
                            nc.vector.tensor_scalar(dd[:], td[:], 0.5, 0.5,
                                                    ALU.mult, ALU.add)
                            dneg = work.tile([P, 4, TB], F32, tag="work", name="dneg")
                            nc.vector.tensor_scalar(dneg[:], td[:], -0.5, 0.5,
                                                    ALU.mult, ALU.add)
                            delta = work.tile([P, 4, TB], F32, tag="work", name="delta")
                            nc.gpsimd.tensor_tensor(delta[:], f2[:], f1[:],
                                                    ALU.subtract)
                            m = work.tile([P, 4, TB], F32, tag="work", name="m")
                            nc.vector.tensor_tensor(m[:], delta[:], tau[:], ALU.mult)
                            nc.gpsimd.tensor_tensor(m[:], m[:], f1[:], ALU.add)
                            cp = work.tile([P, 4, TB], F32, tag="work", name="cp")
                            nc.vector.tensor_tensor(cp[:], m[:], dneg[:], ALU.mult)

                            for lq in range(4):
                                lt = p0 + lq
                                init = 0.0 if c == 0 else h_prev[:, lt, TB - 1:TB]
                                nc.vector.tensor_tensor_scan(
                                    h[:, lt, :], dd[:, lq, :], cp[:, lq, :], init,
                                    ALU.mult, ALU.add)
                        return h

                    def s3_stategate(c, h, zc):
                        """state-out + gating for chunk c -> gh tiles"""
                        ghs = []
                        for hf in range(2):
                            p0 = 4 * hf
                            oseq = work.tile([P, 4, TB], F32, tag="work", name="oseq")
                            stage(5, h, p0, oseq, AF.Identity, 5)
                            gh = work.tile([P, 4, TB], F32R, tag="work", name="gh")
                            nc.gpsimd.tensor_tensor(gh[:], oseq[:],
                                                    zc[:, p0:p0 + 4, :], ALU.mult)
                            ghs.append(gh)
                        return ghs

                    def s4_outproj(c, ghs):
                        """out_proj + store for chunk c"""
                        ysb = pyp.tile([P, TB // P, H], F32, tag="y", name="ysb")
                        for st in range(TB // P):
                            py = psy.tile([P, H], F32, tag="ypsum", name="py")
                            for kt in range(8):
                                lh = ghs[kt // 4][:, kt % 4, st * P:(st + 1) * P]
                                nc.tensor.matmul(
                                    py[:, 0:512], lh, w_out[:, kt, 0:512],
                                    start=(kt == 0), stop=(kt == 7))
                                nc.tensor.matmul(
                                    py[:, 512:1024], lh, w_out[:, kt, 512:1024],
                                    start=(kt == 0), stop=(kt == 7))
                            nc.scalar.activation(ysb[:, st, :], py[:], AF.Copy)
                        nc.sync.dma_start(
                            y_d[c * TB:(c + 1) * TB, :].rearrange(
                                "(st p) j -> p st j", p=P),
                            ysb[:])

                    # Deeper software pipeline:
                    #   iteration c issues: DMA_c, heads+chain of chunk c,
                    #   state-out/gating of chunk c-1, out_proj of chunk c-2.
                    # Every cross-engine dependency is then >= 1 iteration
                    # stale, so the in-order engine streams never stall.
                    h_prev = None
                    so_pend = None   # (c, h, zc)
                    op_pend = None   # (c, ghs)
                    for c in range(NB):
                        xhc = pbs.tile([P, 8, TB], F32R, tag="xhc", name="xhc")
                        nc.sync.dma_start(
                            xhc[:], xh_d[:, :, c * TB:(c + 1) * TB].transpose([1, 0, 2]))
                        zc = pbs.tile([P, 8, TB], F32, tag="zc", name="zc")
                        nc.sync.dma_start(
                            zc[:], zs_d[:, :, c * TB:(c + 1) * TB].transpose([1, 0, 2]))
                        h = s12_heads_chain(c, xhc, h_prev)
                        h_prev = h
                        if so_pend is not None:
                            op_next = (so_pend[0], s3_stategate(*so_pend))
                        else:
                            op_next = None
                        if op_pend is not None:
                            s4_outproj(*op_pend)
                        so_pend = (c, h, zc)
                        op_pend = op_next
                    op_next = (so_pend[0], s3_stategate(*so_pend))
                    if op_pend is not None:
                        s4_outproj(*op_pend)
                    s4_outproj(*op_next)

    nc.compile()
    return nc


def _prep_shared(inputs):
    """Host-side preprocessing of the shared (weight) tensors."""
    f32 = np.float32
    in_proj_w = np.asarray(inputs["in_proj_w"], f32)
    conv_w = np.asarray(inputs["conv_w"], f32)
    conv_b = np.asarray(inputs["conv_b"], f32)

    w_in = in_proj_w.reshape(8, P, 2 * H).transpose(1, 0, 2).copy()
    w_out = np.asarray(inputs["out_proj_w"], f32).reshape(8, P, H).transpose(1, 0, 2).copy()

    cdiag = np.zeros((8, 5, P, P), f32)
    rng = np.arange(P)
    for ct in range(8):
        for tap in range(K):
            cdiag[ct, tap, rng, rng] = conv_w[ct * P:(ct + 1) * P, 0, tap]
        cdiag[ct, 4, rng, rng] = conv_b[ct * P:(ct + 1) * P]
    cdiag = cdiag.transpose(2, 0, 1, 3).copy()  # (P, 8, 5, P)

    def blk2(w):
        o = np.zeros((P, P), f32)
        o[:64, :64] = w
        o[64:, 64:] = w
        return o

    blk = np.stack([
        blk2(np.asarray(inputs["bb_w"], f32)),
        blk2(np.asarray(inputs["f1_w"], f32)),
        blk2(np.asarray(inputs["f2_w"], f32)),
        blk2(np.asarray(inputs["tau_a_w"], f32) * 0.5),
        blk2(np.asarray(inputs["decay_w"], f32) * 0.5),
        blk2(np.asarray(inputs["state_out_w"], f32)),
    ], axis=1)  # (P, 6, P)

    def t2(v):
        return np.tile(np.asarray(v, f32), 2)

    bias = np.stack([
        t2(inputs["bb_b"]),
        t2(inputs["f1_b"]),
        t2(inputs["f2_b"]),
        0.5 * (t2(inputs["tau_a_b"]) + t2(inputs["tau_b"])),
        0.5 * t2(inputs["decay_b"]),
        t2(inputs["state_out_b"]),
    ], axis=1)  # (P, 6)

    ident = np.eye(P, dtype=f32)
    return {
        "w_in": np.ascontiguousarray(w_in),
        "w_out": np.ascontiguousarray(w_out),
        "cdiag": np.ascontiguousarray(cdiag),
        "blk": np.ascontiguousarray(blk),
        "bias": np.ascontiguousarray(bias),
        "ident": ident,
    }


def kernel(**inputs) -> np.ndarray:
    from concourse import bass_utils

    if "nc" not in _CACHE:
        _CACHE["nc"] = _build_program()
    nc = _CACHE["nc"]

    shared = _prep_shared(inputs)
    x = np.asarray(inputs["x"], np.float32)

    in_maps = []
    for b in range(N_CORES):
        m = dict(shared)
        m["x"] = np.ascontiguousarray(x[b])
        in_maps.append(m)

    res = bass_utils.run_bass_kernel_spmd(nc, in_maps, core_ids=list(range(N_CORES)))
    out = np.stack([res.results[b]["y"] for b in range(N_CORES)], axis=0)
    return out.astype(np.float32)


# NOTE on tau/decay: the reference computes
#   tau   = sigmoid(bb @ tau_a_w + tau_a_b + tau_b)
#   decay = sigmoid(bb @ decay_w + decay_b)
# We use sigmoid(u) = 0.5 + 0.5*tanh(u/2): the 0.5 on u is folded into the
# block-diagonal weights (tau_a_w*0.5, decay_w*0.5) and the biases
# (0.5*(tau_a_b+tau_b), 0.5*decay_b); ACT computes tanh(psum + bias) with
# scale 1.0, and the DVE affine 0.5*t + 0.5 recovers the sigmoid.



# revision 5
# speedup vs baseline: 1.5613x; 1.5613x over previous
"""CfC Liquid Cell kernel for Trainium2 (Bass/Tile), 8 NeuronCores.

Sharding: data-parallel over batch (B=8 -> 1 batch element per core).

Single fused pipeline over time chunks of TC=256 (no DRAM scratch):
  per chunk c: DMA x -> PE transpose -> in_proj (bf16) -> depthwise conv
  (diag-matmul taps, bias folded into silu) -> head matmuls (block-diag
  2-head 64x64 weights, shared across heads) -> elementwise gate algebra
  on DVE in bf16 (2x/4x modes) -> tensor_tensor_scan -> state_out ->
  z-gating -> out_proj with gated activations as the stationary operand
  (output produced time-major, DMA'd straight out).

Software pipelining: iteration i issues trans(i+1), in_proj/conv/heads/
scan(i), state_out+gating(i-1), out_proj(i-2) so every cross-engine
dependency has >= 1 chunk of slack.

sigmoid(u) = 0.5 + 0.5*tanh(u/2): the 0.5 on u is folded into the
tau/decay weights+biases on the host; DVE affine recovers sigmoid.
"""

import numpy as np

B, S, H = 8, 2048, 1024
NH, HD, NS, K = 16, 64, 64, 4
N_CORES = 8
P = 128
TC = 256            # time chunk
NC = S // TC        # 8

_CACHE = {}


def _build_program():
    import concourse.bacc as bacc
    import concourse.mybir as mybir
    import concourse.tile as tile

    F32 = mybir.dt.float32
    BF16 = mybir.dt.bfloat16
    AF = mybir.ActivationFunctionType
    ALU = mybir.AluOpType

    nc = bacc.Bacc("TRN2", target_bir_lowering=False, debug=False)

    x_d = nc.dram_tensor("x", (S, H), F32, kind="ExternalInput").ap()
    w_in_d = nc.dram_tensor("w_in", (P, 8, 2 * H), BF16, kind="ExternalInput").ap()
    w_out_d = nc.dram_tensor("w_out", (P, 8, H), BF16, kind="ExternalInput").ap()
    cdiag_d = nc.dram_tensor("cdiag", (P, 8, 4, P), BF16, kind="ExternalInput").ap()
    blk_d = nc.dram_tensor("blk", (P, 6, P), BF16, kind="ExternalInput").ap()
    bias_d = nc.dram_tensor("bias", (P, 6), F32, kind="ExternalInput").ap()
    cbias_d = nc.dram_tensor("cbias", (P, 8), F32, kind="ExternalInput").ap()
    ident_d = nc.dram_tensor("ident", (P, P), F32, kind="ExternalInput").ap()
    y_d = nc.dram_tensor("y", (S, H), F32, kind="ExternalOutput").ap()

    with tile.TileContext(nc) as tc:
        with tc.tile_pool(name="const", bufs=1) as cpool:
            w_in = cpool.tile([P, 8, 2 * H], BF16)
            nc.sync.dma_start(w_in[:], w_in_d[:])
            w_out = cpool.tile([P, 8, H], BF16)
            nc.sync.dma_start(w_out[:], w_out_d[:])
            cdiag = cpool.tile([P, 8, 4, P], BF16)
            nc.sync.dma_start(cdiag[:], cdiag_d[:])
            blk = cpool.tile([P, 6, P], BF16)
            nc.sync.dma_start(blk[:], blk_d[:])
            bias = cpool.tile([P, 6], F32)
            nc.sync.dma_start(bias[:], bias_d[:])
            cbias = cpool.tile([P, 8], F32)
            nc.sync.dma_start(cbias[:], cbias_d[:])
            ident = cpool.tile([P, P], F32)
            nc.sync.dma_start(ident[:], ident_d[:])

            with tc.tile_pool(name="pxc", bufs=3) as pxc, \
                 tc.tile_pool(name="pxT", bufs=2) as pxT, \
                 tc.tile_pool(name="pxp", bufs=2) as pxp, \
                 tc.tile_pool(name="pzs", bufs=3) as pzs, \
                 tc.tile_pool(name="pxh", bufs=2) as pxh, \
                 tc.tile_pool(name="pbb", bufs=2) as pbb, \
                 tc.tile_pool(name="work", bufs=9) as work, \
                 tc.tile_pool(name="ph", bufs=2) as phh, \
                 tc.tile_pool(name="pog", bufs=4) as pog, \
                 tc.tile_pool(name="pyb", bufs=2) as pyb, \
                 tc.tile_pool(name="psA", bufs=3, space="PSUM") as psA, \
                 tc.tile_pool(name="psG", bufs=3, space="PSUM") as psG, \
                 tc.tile_pool(name="psY", bufs=2, space="PSUM") as psY:

                def dma_in(c):
                    xc = pxc.tile([P, 2, H], F32, tag="xc", name="xc")
                    nc.sync.dma_start(
                        xc[:],
                        x_d[c * TC:(c + 1) * TC, :].rearrange(
                            "(ss p) k -> p ss k", p=P))
                    return xc

                def transpose(c, xc):
                    """x chunk -> feature-major bf16 xT"""
                    xT = pxT.tile([P, 8, TC], BF16, tag="xT", name="xT")
                    for kp in range(4):          # pairs of kt
                        pt = psA.tile([P, 4, P], F32, tag="psA", name="pt")
                        n = 0
                        for m in range(2):
                            kt = 2 * kp + m
                            for ss in range(2):
                                nc.tensor.matmul(
                                    pt[:, 2 * m + ss, :],
                                    xc[:, ss, kt * P:(kt + 1) * P],
                                    ident[:], is_transpose=True,
                                    start=(n == 0), stop=(n == 3),
                                    skip_group_check=True)
                                n += 1
                        nc.scalar.activation(xT[:, 2 * kp, :], pt[:, 0:2, :],
                                             AF.Copy)
                        nc.vector.tensor_copy(xT[:, 2 * kp + 1, :],
                                              pt[:, 2:4, :])
                    return xT

                def in_proj(c, xT, xp_prev):
                    """xz = x @ W_in; x_path (jt 0..7) first, then z."""
                    xp = pxp.tile([P, 8, 3 + TC], BF16, tag="xp", name="xp")
                    if c == 0:
                        nc.vector.memset(xp[:, :, :3], 0.0)
                    else:
                        nc.vector.tensor_copy(xp[:, :, :3],
                                              xp_prev[:, :, TC:TC + 3])
                    zs = pzs.tile([P, 8, TC], BF16, tag="zs", name="zs")
                    for jp in range(8):          # pairs of jt
                        jt = 2 * jp
                        pm = psA.tile([P, 2, TC], F32, tag="psA", name="pm")
                        n = 0
                        for kt in range(8):
                            for j in range(2):
                                nc.tensor.matmul(
                                    pm[:, j, :],
                                    w_in[:, kt, (jt + j) * P:(jt + j + 1) * P],
                                    xT[:, kt, :],
                                    start=(n == 0), stop=(n == 15),
                                    skip_group_check=True)
                                n += 1
                        if jp < 4:
                            nc.vector.tensor_copy(xp[:, jt:jt + 2, 3:], pm[:])
                        else:
                            nc.scalar.activation(zs[:, jt - 8:jt - 6, :],
                                                 pm[:], AF.Silu)
                    return xp, zs

                def conv(c, xp):
                    """depthwise causal conv + silu (bias in the act)."""
                    xh = pxh.tile([P, 8, TC], BF16, tag="xh", name="xh")
                    for cp in range(4):          # pairs of ct
                        ct = 2 * cp
                        pc = psA.tile([P, 2, TC], F32, tag="psA", name="pc")
                        n = 0
                        for tap in range(4):
                            for j in range(2):
                                nc.tensor.matmul(
                                    pc[:, j, :], cdiag[:, ct + j, tap, :],
                                    xp[:, ct + j, tap:tap + TC],
                                    start=(n == 0), stop=(n == 7),
                                    skip_group_check=True)
                                n += 1
                        for j in range(2):
                            nc.scalar.activation(
                                xh[:, ct + j, :], pc[:, j, :], AF.Silu,
                                bias=cbias[:, ct + j:ct + j + 1])
                    return xh

                def stage(widx, rhs_t, out_t, func, bias_col):
                    for q in range(4):
                        pg = psG.tile([P, 2, TC], F32, tag="psG", name="pg")
                        nc.tensor.matmul(
                            pg[:], blk[:, widx, :],
                            rhs_t[:, 2 * q:2 * q + 2, :],
                            start=True, stop=True)
                        nc.scalar.activation(
                            out_t[:, 2 * q:2 * q + 2, :], pg[:], func,
                            bias=bias[:, bias_col:bias_col + 1])

                def heads_scan(c, xh, h_prev):
                    bb = pbb.tile([P, 8, TC], BF16, tag="bb", name="bb")
                    stage(0, xh, bb, AF.Silu, 0)
                    f1 = work.tile([P, 8, TC], BF16, tag="work", name="f1")
                    stage(1, bb, f1, AF.Tanh, 1)
                    f2 = work.tile([P, 8, TC], BF16, tag="work", name="f2")
                    stage(2, bb, f2, AF.Tanh, 2)
                    ta = work.tile([P, 8, TC], BF16, tag="work", name="ta")
                    stage(3, bb, ta, AF.Tanh, 3)
                    tg = work.tile([P, 8, TC], BF16, tag="work", name="tg")
                    stage(4, bb, tg, AF.Tanh, 4)

                    # candidate*2 = (f1+f2) + a*(f2-f1); u = c2 * (1-g)/4
                    sm = work.tile([P, 8, TC], BF16, tag="work", name="sm")
                    nc.vector.tensor_tensor(sm[:], f1[:], f2[:], ALU.add)
                    dl = work.tile([P, 8, TC], BF16, tag="work", name="dl")
                    nc.vector.tensor_tensor(dl[:], f2[:], f1[:], ALU.subtract)
                    tt = work.tile([P, 8, TC], BF16, tag="work", name="tt")
                    nc.vector.tensor_tensor(tt[:], ta[:], dl[:], ALU.mult)
                    c2 = work.tile([P, 8, TC], BF16, tag="work", name="c2")
                    nc.vector.tensor_tensor(c2[:], sm[:], tt[:], ALU.add)
                    wq = work.tile([P, 8, TC], BF16, tag="work", name="wq")
                    nc.vector.tensor_scalar(wq[:], tg[:], -0.25, 0.25,
                                            ALU.mult, ALU.add)
                    uu = work.tile([P, 8, TC], BF16, tag="work", name="uu")
                    nc.vector.tensor_tensor(uu[:], c2[:], wq[:], ALU.mult)
                    dd = work.tile([P, 8, TC], BF16, tag="work", name="dd")
                    nc.vector.tensor_scalar(dd[:], tg[:], 0.5, 0.5,
                                            ALU.mult, ALU.add)

                    h = phh.tile([P, 8, TC], BF16, tag="h", name="h")
                    for hp in range(8):
                        init = 0.0 if c == 0 else h_prev[:, hp, TC - 1:TC]
                        nc.vector.tensor_tensor_scan(
                            h[:, hp, :], dd[:, hp, :], uu[:, hp, :], init,
                            ALU.mult, ALU.add)
                    return h

                def stategate(c, h, zs):
                    oseq = pog.tile([P, 8, TC], BF16, tag="og", name="oseq")
                    stage(5, h, oseq, AF.Identity, 5)
                    gh = pog.tile([P, 8, TC], BF16, tag="og", name="gh")
                    nc.gpsimd.tensor_tensor(gh[:], oseq[:], zs[:], ALU.mult)
                    return gh

                def out_proj(c, gh):
                    for tb in range(TC // P):
                        ysb = pyb.tile([P, H], F32, tag="ysb", name="ysb")
                        for hf in range(2):
                            py = psY.tile([P, H // 2], F32, tag="psY", name="py")
                            for kt in range(8):
                                nc.tensor.matmul(
                                    py[:], gh[:, kt, tb * P:(tb + 1) * P],
                                    w_out[:, kt, hf * 512:(hf + 1) * 512],
                                    start=(kt == 0), stop=(kt == 7))
                            nc.scalar.activation(
                                ysb[:, hf * 512:(hf + 1) * 512], py[:], AF.Copy)
                        nc.sync.dma_start(
                            y_d[(c * 2 + tb) * P:(c * 2 + tb + 1) * P, :],
                            ysb[:])

                # software pipeline; x DMA prefetched 2 chunks ahead
                xcs = {0: dma_in(0), 1: dma_in(1)}
                xT_cur = transpose(0, xcs.pop(0))
                xp_prev = None
                h_prev = None
                sg_pend = None   # (c, h, zs)
                op_pend = None   # (c, gh)
                for i in range(NC + 2):
                    if i + 2 < NC:
                        xcs[i + 2] = dma_in(i + 2)
                    if i + 1 < NC:
                        xT_next = transpose(i + 1, xcs.pop(i + 1))
                    else:
                        xT_next = None
                    if i < NC:
                        xp, zs = in_proj(i, xT_cur, xp_prev)
                        xp_prev = xp
                        xh = conv(i, xp)
                        h = heads_scan(i, xh, h_prev)
                        h_prev = h
                        sg_next = (i, h, zs)
                    else:
                        sg_next = None
                    if sg_pend is not None:
                        op_next = (sg_pend[0], stategate(sg_pend[0],
                                                         sg_pend[1], sg_pend[2]))
                    else:
                        op_next = None
                    if op_pend is not None:
                        out_proj(*op_pend)
                    sg_pend = sg_next
                    op_pend = op_next
                    xT_cur = xT_next

    nc.compile()
    return nc


def _prep_shared(inputs):
    """Host-side preprocessing of the shared (weight) tensors."""
    import ml_dtypes
    f32 = np.float32
    bf = ml_dtypes.bfloat16
    in_proj_w = np.asarray(inputs["in_proj_w"], f32)
    conv_w = np.asarray(inputs["conv_w"], f32)
    conv_b = np.asarray(inputs["conv_b"], f32)

    w_in = in_proj_w.reshape(8, P, 2 * H).transpose(1, 0, 2)
    w_out = np.asarray(inputs["out_proj_w"], f32).reshape(8, P, H).transpose(1, 0, 2)

    cdiag = np.zeros((8, 4, P, P), f32)
    rng = np.arange(P)
    for ct in range(8):
        for tap in range(K):
            cdiag[ct, tap, rng, rng] = conv_w[ct * P:(ct + 1) * P, 0, tap]
    cdiag = cdiag.transpose(2, 0, 1, 3)  # (P, 8, 4, P)
    cbias = conv_b.reshape(8, P).T  # (P, 8)

    def blk2(w):
        o = np.zeros((P, P), f32)
        o[:64, :64] = w
        o[64:, 64:] = w
        return o

    blk = np.stack([
        blk2(np.asarray(inputs["bb_w"], f32)),
        blk2(np.asarray(inputs["f1_w"], f32)),
        blk2(np.asarray(inputs["f2_w"], f32)),
        blk2(np.asarray(inputs["tau_a_w"], f32) * 0.5),
        blk2(np.asarray(inputs["decay_w"], f32) * 0.5),
        blk2(np.asarray(inputs["state_out_w"], f32)),
    ], axis=1)  # (P, 6, P)

    def t2(v):
        return np.tile(np.asarray(v, f32), 2)

    bias = np.stack([
        t2(inputs["bb_b"]),
        t2(inputs["f1_b"]),
        t2(inputs["f2_b"]),
        0.5 * (t2(inputs["tau_a_b"]) + t2(inputs["tau_b"])),
        0.5 * t2(inputs["decay_b"]),
        t2(inputs["state_out_b"]),
    ], axis=1)  # (P, 6)

    ident = np.eye(P, dtype=f32)
    return {
        "w_in": np.ascontiguousarray(w_in).astype(bf),
        "w_out": np.ascontiguousarray(w_out).astype(bf),
        "cdiag": np.ascontiguousarray(cdiag).astype(bf),
        "blk": np.ascontiguousarray(blk).astype(bf),
        "bias": np.ascontiguousarray(bias),
        "cbias": np.ascontiguousarray(cbias),
        "ident": ident,
    }


def kernel(**inputs) -> np.ndarray:
    from concourse import bass_utils

    if "nc" not in _CACHE:
        _CACHE["nc"] = _build_program()
    nc = _CACHE["nc"]

    shared = _prep_shared(inputs)
    x = np.asarray(inputs["x"], np.float32)

    in_maps = []
    for b in range(N_CORES):
        m = dict(shared)
        m["x"] = np.ascontiguousarray(x[b])
        in_maps.append(m)

    res = bass_utils.run_bass_kernel_spmd(nc, in_maps, core_ids=list(range(N_CORES)))
    out = np.stack([res.results[b]["y"] for b in range(N_CORES)], axis=0)
    return out.astype(np.float32)


# revision 15
# speedup vs baseline: 1.8431x; 1.1805x over previous
"""CfC Liquid Cell kernel for Trainium2 (Bass/Tile), 8 NeuronCores.

Sharding: data-parallel over batch (B=8 -> 1 batch element per core).

Single fused pipeline over time chunks of TC=256 (no DRAM scratch):
  per chunk c: DMA x -> PE transpose -> in_proj (bf16) -> depthwise conv
  (diag-matmul taps, bias folded into silu) -> head matmuls (block-diag
  2-head 64x64 weights, shared across heads) -> elementwise gate algebra
  on DVE in bf16 (2x/4x modes) -> tensor_tensor_scan -> state_out ->
  z-gating -> out_proj with gated activations as the stationary operand
  (output produced time-major, DMA'd straight out).

Software pipelining: iteration i issues trans(i+1), in_proj/conv/heads/
scan(i), state_out+gating(i-1), out_proj(i-2) so every cross-engine
dependency has >= 1 chunk of slack.

sigmoid(u) = 0.5 + 0.5*tanh(u/2): the 0.5 on u is folded into the
tau/decay weights+biases on the host; DVE affine recovers sigmoid.
"""

import numpy as np

B, S, H = 8, 2048, 1024
NH, HD, NS, K = 16, 64, 64, 4
N_CORES = 8
P = 128
TC = 256            # time chunk
NC = S // TC        # 8

_CACHE = {}


def _build_program():
    import concourse.bacc as bacc
    import concourse.mybir as mybir
    import concourse.tile as tile

    F32 = mybir.dt.float32
    BF16 = mybir.dt.bfloat16
    AF = mybir.ActivationFunctionType
    ALU = mybir.AluOpType

    nc = bacc.Bacc("TRN2", target_bir_lowering=False, debug=False)

    x_d = nc.dram_tensor("x", (S, H), BF16, kind="ExternalInput").ap()
    w_in_d = nc.dram_tensor("w_in", (P, 8, 2 * H), BF16, kind="ExternalInput").ap()
    w_out_d = nc.dram_tensor("w_out", (P, 8, H), BF16, kind="ExternalInput").ap()
    cdiag_d = nc.dram_tensor("cdiag", (P, 8, 4, P), BF16, kind="ExternalInput").ap()
    blk_d = nc.dram_tensor("blk", (P, 6, P), BF16, kind="ExternalInput").ap()
    bias_d = nc.dram_tensor("bias", (P, 6), F32, kind="ExternalInput").ap()
    cbias_d = nc.dram_tensor("cbias", (P, 8), F32, kind="ExternalInput").ap()
    y_d = nc.dram_tensor("y", (S, H), F32, kind="ExternalOutput").ap()

    with tile.TileContext(nc) as tc:
        with tc.tile_pool(name="const", bufs=1) as cpool:
            w_in = cpool.tile([P, 8, 2 * H], BF16)
            nc.sync.dma_start(w_in[:], w_in_d[:])
            w_out = cpool.tile([P, 8, H], BF16)
            nc.sync.dma_start(w_out[:], w_out_d[:])
            cdiag = cpool.tile([P, 8, 4, P], BF16)
            nc.sync.dma_start(cdiag[:], cdiag_d[:])
            blk = cpool.tile([P, 6, P], BF16)
            nc.sync.dma_start(blk[:], blk_d[:])
            bias = cpool.tile([P, 6], F32)
            nc.sync.dma_start(bias[:], bias_d[:])
            cbias = cpool.tile([P, 8], F32)
            nc.sync.dma_start(cbias[:], cbias_d[:])

            with tc.tile_pool(name="pxT", bufs=3) as pxT, \
                 tc.tile_pool(name="pxp", bufs=2) as pxp, \
                 tc.tile_pool(name="pzs", bufs=3) as pzs, \
                 tc.tile_pool(name="pxh", bufs=2) as pxh, \
                 tc.tile_pool(name="pbb", bufs=2) as pbb, \
                 tc.tile_pool(name="work", bufs=9) as work, \
                 tc.tile_pool(name="ph", bufs=2) as phh, \
                 tc.tile_pool(name="pog", bufs=4) as pog, \
                 tc.tile_pool(name="pyb", bufs=2) as pyb, \
                 tc.tile_pool(name="psA", bufs=3, space="PSUM") as psA, \
                 tc.tile_pool(name="psG", bufs=3, space="PSUM") as psG, \
                 tc.tile_pool(name="psY", bufs=2, space="PSUM") as psY:

                def dma_in(c):
                    """XBAR DMA transpose: x chunk -> feature-major bf16 xT"""
                    xT = pxT.tile([P, 8, TC], BF16, tag="xT", name="xT")
                    nc.sync.dma_start_transpose(
                        xT[:], x_d[c * TC:(c + 1) * TC, :])
                    return xT

                def in_proj(c, xT, xp_prev):
                    """xz = x @ W_in; x_path (jt 0..7) first, then z."""
                    xp = pxp.tile([P, 8, 3 + TC], BF16, tag="xp", name="xp")
                    if c == 0:
                        nc.vector.memset(xp[:, :, :3], 0.0)
                    else:
                        nc.vector.tensor_copy(xp[:, :, :3],
                                              xp_prev[:, :, TC:TC + 3])
                    zs = pzs.tile([P, 8, TC], BF16, tag="zs", name="zs")
                    for jp in range(8):          # pairs of jt
                        jt = 2 * jp
                        pm = psA.tile([P, 2, TC], F32, tag="psA", name="pm")
                        n = 0
                        for kt in range(8):
                            for j in range(2):
                                nc.tensor.matmul(
                                    pm[:, j, :],
                                    w_in[:, kt, (jt + j) * P:(jt + j + 1) * P],
                                    xT[:, kt, :],
                                    start=(n == 0), stop=(n == 15),
                                    skip_group_check=True)
                                n += 1
                        if jp < 4:
                            nc.vector.tensor_copy(xp[:, jt:jt + 2, 3:], pm[:])
                        else:
                            nc.scalar.activation(zs[:, jt - 8:jt - 6, :],
                                                 pm[:], AF.Silu)
                    return xp, zs

                def conv(c, xp):
                    """depthwise causal conv + silu (bias in the act)."""
                    xh = pxh.tile([P, 8, TC], BF16, tag="xh", name="xh")
                    for cp in range(4):          # pairs of ct
                        ct = 2 * cp
                        pc = psA.tile([P, 2, TC], F32, tag="psA", name="pc")
                        n = 0
                        for tap in range(4):
                            for j in range(2):
                                nc.tensor.matmul(
                                    pc[:, j, :], cdiag[:, ct + j, tap, :],
                                    xp[:, ct + j, tap:tap + TC],
                                    start=(n == 0), stop=(n == 7),
                                    skip_group_check=True)
                                n += 1
                        for j in range(2):
                            nc.scalar.activation(
                                xh[:, ct + j, :], pc[:, j, :], AF.Silu,
                                bias=cbias[:, ct + j:ct + j + 1])
                    return xh

                def stage(widx, rhs_t, out_t, func, bias_col):
                    for q in range(4):
                        pg = psG.tile([P, 2, TC], F32, tag="psG", name="pg")
                        nc.tensor.matmul(
                            pg[:], blk[:, widx, :],
                            rhs_t[:, 2 * q:2 * q + 2, :],
                            start=True, stop=True)
                        nc.scalar.activation(
                            out_t[:, 2 * q:2 * q + 2, :], pg[:], func,
                            bias=bias[:, bias_col:bias_col + 1])

                def heads_scan(c, xh, h_prev):
                    bb = pbb.tile([P, 8, TC], BF16, tag="bb", name="bb")
                    stage(0, xh, bb, AF.Silu, 0)
                    f1 = work.tile([P, 8, TC], BF16, tag="work", name="f1")
                    stage(1, bb, f1, AF.Tanh, 1)
                    f2 = work.tile([P, 8, TC], BF16, tag="work", name="f2")
                    stage(2, bb, f2, AF.Tanh, 2)
                    ta = work.tile([P, 8, TC], BF16, tag="work", name="ta")
                    stage(3, bb, ta, AF.Tanh, 3)
                    tg = work.tile([P, 8, TC], BF16, tag="work", name="tg")
                    stage(4, bb, tg, AF.Tanh, 4)

                    # candidate*2 = (f1+f2) + a*(f2-f1); u = c2 * (1-g)/4
                    sm = work.tile([P, 8, TC], BF16, tag="work", name="sm")
                    nc.vector.tensor_tensor(sm[:], f1[:], f2[:], ALU.add)
                    dl = work.tile([P, 8, TC], BF16, tag="work", name="dl")
                    nc.vector.tensor_tensor(dl[:], f2[:], f1[:], ALU.subtract)
                    tt = work.tile([P, 8, TC], BF16, tag="work", name="tt")
                    nc.vector.tensor_tensor(tt[:], ta[:], dl[:], ALU.mult)
                    c2 = work.tile([P, 8, TC], BF16, tag="work", name="c2")
                    nc.vector.tensor_tensor(c2[:], sm[:], tt[:], ALU.add)
                    wq = work.tile([P, 8, TC], BF16, tag="work", name="wq")
                    nc.vector.tensor_scalar(wq[:], tg[:], -0.25, 0.25,
                                            ALU.mult, ALU.add)
                    uu = work.tile([P, 8, TC], BF16, tag="work", name="uu")
                    nc.vector.tensor_tensor(uu[:], c2[:], wq[:], ALU.mult)
                    dd = work.tile([P, 8, TC], BF16, tag="work", name="dd")
                    nc.vector.tensor_scalar(dd[:], tg[:], 0.5, 0.5,
                                            ALU.mult, ALU.add)

                    h = phh.tile([P, 8, TC], BF16, tag="h", name="h")
                    for hp in range(8):
                        init = 0.0 if c == 0 else h_prev[:, hp, TC - 1:TC]
                        nc.vector.tensor_tensor_scan(
                            h[:, hp, :], dd[:, hp, :], uu[:, hp, :], init,
                            ALU.mult, ALU.add)
                    return h

                def stategate(c, h, zs):
                    oseq = pog.tile([P, 8, TC], BF16, tag="og", name="oseq")
                    stage(5, h, oseq, AF.Identity, 5)
                    gh = pog.tile([P, 8, TC], BF16, tag="og", name="gh")
                    nc.gpsimd.tensor_tensor(gh[:], oseq[:], zs[:], ALU.mult)
                    return gh

                def out_proj(c, gh):
                    for tb in range(TC // P):
                        ysb = pyb.tile([P, H], F32, tag="ysb", name="ysb")
                        for hf in range(2):
                            py = psY.tile([P, H // 2], F32, tag="psY", name="py")
                            for kt in range(8):
                                nc.tensor.matmul(
                                    py[:], gh[:, kt, tb * P:(tb + 1) * P],
                                    w_out[:, kt, hf * 512:(hf + 1) * 512],
                                    start=(kt == 0), stop=(kt == 7))
                            if hf == 0:
                                nc.scalar.activation(
                                    ysb[:, 0:512], py[:], AF.Copy)
                            else:
                                nc.vector.tensor_copy(ysb[:, 512:1024], py[:])
                        nc.sync.dma_start(
                            y_d[(c * 2 + tb) * P:(c * 2 + tb + 1) * P, :],
                            ysb[:])

                # software pipeline; x DMA-transpose prefetched 1 ahead
                xT_cur = dma_in(0)
                xp_prev = None
                h_prev = None
                sg_pend = None   # (c, h, zs)
                op_pend = None   # (c, gh)
                for i in range(NC + 2):
                    if i + 1 < NC:
                        xT_next = dma_in(i + 1)
                    else:
                        xT_next = None
                    if i < NC:
                        xp, zs = in_proj(i, xT_cur, xp_prev)
                        xp_prev = xp
                        xh = conv(i, xp)
                        h = heads_scan(i, xh, h_prev)
                        h_prev = h
                        sg_next = (i, h, zs)
                    else:
                        sg_next = None
                    if sg_pend is not None:
                        op_next = (sg_pend[0], stategate(sg_pend[0],
                                                         sg_pend[1], sg_pend[2]))
                    else:
                        op_next = None
                    if op_pend is not None:
                        out_proj(*op_pend)
                    sg_pend = sg_next
                    op_pend = op_next
                    xT_cur = xT_next

    nc.compile()
    return nc


def _prep_shared(inputs):
    """Host-side preprocessing of the shared (weight) tensors."""
    import ml_dtypes
    f32 = np.float32
    bf = ml_dtypes.bfloat16
    in_proj_w = np.asarray(inputs["in_proj_w"], f32)
    conv_w = np.asarray(inputs["conv_w"], f32)
    conv_b = np.asarray(inputs["conv_b"], f32)

    w_in = in_proj_w.reshape(8, P, 2 * H).transpose(1, 0, 2)
    w_out = np.asarray(inputs["out_proj_w"], f32).reshape(8, P, H).transpose(1, 0, 2)

    cdiag = np.zeros((8, 4, P, P), f32)
    rng = np.arange(P)
    for ct in range(8):
        for tap in range(K):
            cdiag[ct, tap, rng, rng] = conv_w[ct * P:(ct + 1) * P, 0, tap]
    cdiag = cdiag.transpose(2, 0, 1, 3)  # (P, 8, 4, P)
    cbias = conv_b.reshape(8, P).T  # (P, 8)

    def blk2(w):
        o = np.zeros((P, P), f32)
        o[:64, :64] = w
        o[64:, 64:] = w
        return o

    blk = np.stack([
        blk2(np.asarray(inputs["bb_w"], f32)),
        blk2(np.asarray(inputs["f1_w"], f32)),
        blk2(np.asarray(inputs["f2_w"], f32)),
        blk2(np.asarray(inputs["tau_a_w"], f32) * 0.5),
        blk2(np.asarray(inputs["decay_w"], f32) * 0.5),
        blk2(np.asarray(inputs["state_out_w"], f32)),
    ], axis=1)  # (P, 6, P)

    def t2(v):
        return np.tile(np.asarray(v, f32), 2)

    bias = np.stack([
        t2(inputs["bb_b"]),
        t2(inputs["f1_b"]),
        t2(inputs["f2_b"]),
        0.5 * (t2(inputs["tau_a_b"]) + t2(inputs["tau_b"])),
        0.5 * t2(inputs["decay_b"]),
        t2(inputs["state_out_b"]),
    ], axis=1)  # (P, 6)

    return {
        "w_in": np.ascontiguousarray(w_in).astype(bf),
        "w_out": np.ascontiguousarray(w_out).astype(bf),
        "cdiag": np.ascontiguousarray(cdiag).astype(bf),
        "blk": np.ascontiguousarray(blk).astype(bf),
        "bias": np.ascontiguousarray(bias),
        "cbias": np.ascontiguousarray(cbias),
    }


def _in_maps(inputs):
    import ml_dtypes
    shared = _prep_shared(inputs)
    x = np.asarray(inputs["x"], np.float32).astype(ml_dtypes.bfloat16)
    in_maps = []
    for b in range(N_CORES):
        m = dict(shared)
        m["x"] = np.ascontiguousarray(x[b])
        in_maps.append(m)
    return in_maps


def kernel(**inputs) -> np.ndarray:
    from concourse import bass_utils

    if "nc" not in _CACHE:
        _CACHE["nc"] = _build_program()
    nc = _CACHE["nc"]

    res = bass_utils.run_bass_kernel_spmd(nc, _in_maps(inputs),
                                          core_ids=list(range(N_CORES)))
    out = np.stack([res.results[b]["y"] for b in range(N_CORES)], axis=0)
    return out.astype(np.float32)
